# revision 29
# baseline (speedup 1.0000x reference)
"""Trainium2 Bass kernel for nn_AdaptiveAdjacency: cosine-similarity top-k.

kernel(embeddings: [16384, 128] f32) -> (values [16384, 20] f32,
                                         indices [16384, 20] int32)

Strategy (8 NeuronCores, SPMD): embeddings are replicated to every core;
core i computes rows [2048*i, 2048*(i+1)) of the similarity matrix against
all 16384 columns in fp32 on the TensorEngine (chunks of 512 columns into
PSUM), and selects per-row top-k on-device:
  - per 512-chunk top-8 values (VectorE MAX8) + in-chunk positions
    (FIND_INDEX8) form 256 exact-f32 candidates per row,
  - a 3-round max8/max_index/match_replace merge picks the top-24 with
    jax.lax.top_k tie semantics (descending value, ascending index),
  - two GPSIMD per-partition local_scatters convert candidate positions
    into value-ranked global indices without any host-side selection,
  - a per-row flag marks rows where one chunk held >= 9 of the row's
    top-20 (candidate filter insufficient); the host recomputes exactly
    those rows (probability ~1e-7 per chunk => ~0 rows expected).
"""

import os
from contextlib import ExitStack

import numpy as np

import concourse.bass as bass
import concourse.mybir as mybir
from concourse import bacc
from concourse.tile import TileContext
from concourse.masks import make_identity
from concourse.bass_utils import run_bass_kernel_spmd

F32 = mybir.dt.float32
U16 = mybir.dt.uint16
I16 = mybir.dt.int16

N = 16384
D = 128
NC = 8
R = N // NC  # rows per core
K = 20
KPAD = 24
CHUNK = 512
MMW = 512
WIN = 1024  # DVE top-8 window (SBUF); >=9-of-top-20 per window falls back
IMM_LO = -3.0


def _build(num_devices=NC, mm_bufs=6, cand_bufs=3, stage2_at=8):
    NCHUNK = N // CHUNK
    NWIN = N // WIN
    WPC = WIN // CHUNK
    NCAND = NWIN * 8
    assert NCAND >= KPAD
    ROWTILES = R // 128
    NTILE = N // 128
    TPG = 8
    DUMP = NCAND + 32

    nc = bacc.Bacc("TRN2", target_bir_lowering=False, debug=False,
                   num_devices=num_devices)
    emb = nc.dram_tensor("emb", [N, D], F32, kind="ExternalInput").ap()
    slab = nc.dram_tensor("slab", [R, D], F32, kind="ExternalInput").ap()
    out_vals = nc.dram_tensor("out_vals", [R, KPAD], F32,
                              kind="ExternalOutput").ap()
    out_idx = nc.dram_tensor("out_idx", [R, KPAD], U16,
                             kind="ExternalOutput").ap()
    out_flag = nc.dram_tensor("out_flag", [R, 1], F32,
                              kind="ExternalOutput").ap()

    with TileContext(nc) as tc, ExitStack() as ctx:
        const_pool = ctx.enter_context(tc.tile_pool(name="const", bufs=1))
        big_pool = ctx.enter_context(tc.tile_pool(name="big", bufs=1))
        norm_pool = ctx.enter_context(tc.tile_pool(name="norm", bufs=10))
        tp_psum = ctx.enter_context(tc.tile_pool(name="tpps", bufs=2,
                                                 space="PSUM"))
        mm_psum = ctx.enter_context(tc.tile_pool(name="mmps", bufs=mm_bufs,
                                                 space="PSUM"))
        cand_pool = ctx.enter_context(tc.tile_pool(name="cand",
                                                   bufs=cand_bufs))
        sbc_pool = ctx.enter_context(tc.tile_pool(name="sbc", bufs=8))
        s2_pool = ctx.enter_context(tc.tile_pool(name="s2", bufs=2))

        ident = const_pool.tile([128, 128], F32)
        make_identity(nc, ident[:])
        offs16 = const_pool.tile([128, NCAND], U16)
        nc.gpsimd.iota(offs16[:], pattern=[[WIN, NWIN], [0, 8]],
                       base=0, channel_multiplier=0)
        dump16 = const_pool.tile([128, NCAND], U16)
        nc.gpsimd.iota(dump16[:], pattern=[[1, NCAND]], base=32,
                       channel_multiplier=0)
        dump_f = const_pool.tile([128, NCAND], F32)
        nc.vector.tensor_copy(dump_f[:], dump16[:])
        iota24 = const_pool.tile([128, KPAD], U16)
        nc.gpsimd.iota(iota24[:], pattern=[[1, KPAD]], base=1,
                       channel_multiplier=0)

        normT = big_pool.tile([128, N], F32)
        rowT = big_pool.tile([128, R], F32)

        def norm_group(src_ap, g, ntiles_left, dstT):
            n = min(TPG, ntiles_left)
            ssq = norm_pool.tile([128, TPG], F32, tag="ssq")
            et4 = norm_pool.tile([128, TPG, D], F32, tag="et4")
            src3 = src_ap.rearrange("(t p) d -> p t d", p=128)
            nc.sync.dma_start(out=et4[:, :n, :],
                              in_=src3[:, g * TPG:g * TPG + n, :])
            ets = []
            for j in range(n):
                et = et4[:, j, :]
                trash = norm_pool.tile([128, D], F32, tag="trash")
                if j % 2 == 0:
                    nc.scalar.activation(trash[:], et,
                                         mybir.ActivationFunctionType.Square,
                                         accum_out=ssq[:, j:j + 1])
                else:
                    nc.vector.tensor_tensor(out=trash[:], in0=et,
                                            in1=et,
                                            op=mybir.AluOpType.mult)
                    nc.vector.reduce_sum(ssq[:, j:j + 1], trash[:],
                                         axis=mybir.AxisListType.X)
                ets.append(et)
            rt = norm_pool.tile([128, TPG], F32, tag="rt")
            nc.scalar.sqrt(rt[:, :n], ssq[:, :n])
            nc.vector.reciprocal(rt[:, :n], rt[:, :n])
            # one Newton step for rsqrt: r' = 0.5*r*(3 - ssq*r^2)
            r2 = norm_pool.tile([128, TPG], F32, tag="r2")
            nc.vector.tensor_tensor(out=r2[:, :n], in0=rt[:, :n],
                                    in1=rt[:, :n], op=mybir.AluOpType.mult)
            nc.vector.tensor_tensor(out=r2[:, :n], in0=ssq[:, :n],
                                    in1=r2[:, :n], op=mybir.AluOpType.mult)
            # out = (b - 3) * r, then * -0.5  =>  0.5*r*(3-b)
            nc.vector.scalar_tensor_tensor(
                out=r2[:, :n], in0=r2[:, :n], scalar=3.0, in1=rt[:, :n],
                op0=mybir.AluOpType.subtract, op1=mybir.AluOpType.mult)
            nc.vector.tensor_scalar_mul(rt[:, :n], r2[:, :n], -0.5)
            for j in range(n):
                i = g * TPG + j
                nt = norm_pool.tile([128, D], F32, tag=f"nt{j}")
                if j % 2 == 0:
                    nc.scalar.mul(nt[:], ets[j], rt[:, j:j + 1])
                else:
                    nc.vector.tensor_scalar_mul(nt[:], ets[j],
                                                rt[:, j:j + 1])
                pt = tp_psum.tile([128, 128], F32)
                nc.tensor.transpose(pt[:], nt[:], ident[:])
                if j % 2 == 0:
                    nc.scalar.copy(dstT[:, i * 128:(i + 1) * 128], pt[:])
                else:
                    nc.vector.tensor_copy(dstT[:, i * 128:(i + 1) * 128],
                                          pt[:])

        for g in range((R // 128 + TPG - 1) // TPG):
            norm_group(slab, g, ROWTILES - g * TPG, rowT)
        for g in range((NTILE + TPG - 1) // TPG):
            norm_group(emb, g, NTILE - g * TPG, normT)

        def emit_stage2_a(st):
            (m, cand3, cand, cpos) = st
            gidx = s2_pool.tile([128, NCAND], U16, tag="gidx")
            nc.vector.tensor_tensor(out=gidx[:], in0=cpos, in1=offs16[:],
                                    op=mybir.AluOpType.add)
            v8 = s2_pool.tile([128, NWIN], F32, tag="v8")
            nc.vector.tensor_copy(v8[:], cand3[:, :, 7])
            wv = s2_pool.tile([128, KPAD], F32, tag="wv")
            wp = s2_pool.tile([128, KPAD], U16, tag="wp")
            for r in range(KPAD // 8):
                sl = slice(r * 8, (r + 1) * 8)
                nc.vector.max(wv[:, sl], cand)
                nc.vector.max_index(wp[:, sl], wv[:, sl], cand)
                if r < KPAD // 8 - 1:
                    nc.vector.match_replace(cand, wv[:, sl], cand, IMM_LO)
            mv8 = s2_pool.tile([128, 1], F32, tag="mv8")
            nc.vector.tensor_reduce(mv8[:], v8[:], axis=mybir.AxisListType.X,
                                    op=mybir.AluOpType.max)
            flag = s2_pool.tile([128, 1], F32, tag="flag")
            nc.vector.tensor_tensor(out=flag[:], in0=mv8[:],
                                    in1=wv[:, K - 1:K],
                                    op=mybir.AluOpType.is_ge)
            vr16 = s2_pool.tile([128, NCAND], U16, tag="vr16")
            nc.gpsimd.local_scatter(vr16[:], iota24[:], wp[:].bitcast(I16),
                                    channels=128, num_elems=NCAND,
                                    num_idxs=KPAD)
            return (m, gidx, wv, flag, vr16)

        def emit_stage2_b(stb, eng=None):
            (m, gidx, wv, flag, vr16) = stb
            eng = eng or nc.gpsimd
            vr_f = s2_pool.tile([128, NCAND], F32, tag="vr_f")
            eng.tensor_copy(vr_f[:], vr16[:])
            marked = s2_pool.tile([128, NCAND], F32, tag="marked")
            eng.tensor_scalar(out=marked[:], in0=vr_f[:], scalar1=0.0,
                                    scalar2=None,
                                    op0=mybir.AluOpType.is_gt)
            t0 = s2_pool.tile([128, NCAND], F32, tag="t0")
            eng.tensor_tensor(out=t0[:], in0=marked[:], in1=dump_f[:],
                                    op=mybir.AluOpType.mult)
            eng.tensor_tensor(out=t0[:], in0=dump_f[:], in1=t0[:],
                                    op=mybir.AluOpType.subtract)
            eng.tensor_tensor(out=vr_f[:], in0=vr_f[:], in1=t0[:],
                                    op=mybir.AluOpType.add)
            idx2 = s2_pool.tile([128, NCAND], I16, tag="idx2")
            eng.tensor_copy(idx2[:], vr_f[:])
            gbr = s2_pool.tile([128, DUMP], U16, tag="gbr")
            nc.gpsimd.local_scatter(gbr[:], gidx[:], idx2[:],
                                    channels=128, num_elems=DUMP,
                                    num_idxs=NCAND)
            rs = slice(m * 128, (m + 1) * 128)
            nc.sync.dma_start(out=out_vals[rs, :], in_=wv[:])
            nc.sync.dma_start(out=out_idx[rs, :], in_=gbr[:, 1:KPAD + 1])
            nc.sync.dma_start(out=out_flag[rs, :], in_=flag[:])

        s2at = min(stage2_at, NWIN - 1)
        s2bt = min(stage2_at + 5, NWIN - 1)
        pending = None
        pend_b = None
        for m in range(ROWTILES):
            lhsT = rowT[:, m * 128:(m + 1) * 128]
            cand3 = cand_pool.tile([128, NWIN, 8], F32, tag="cand")
            cpos3 = cand_pool.tile([128, NWIN, 8], U16, tag="cpos")
            cand = cand3[:].rearrange("p c e -> p (c e)")
            cpos = cpos3[:].rearrange("p c e -> p (c e)")
            for c in range(NWIN):
                sbc = sbc_pool.tile([128, WIN], F32, tag="sbc")
                for w in range(WPC):
                    ps = mm_psum.tile([128, CHUNK], F32)
                    for v in range(CHUNK // MMW):
                        lo = c * WIN + w * CHUNK + v * MMW
                        nc.tensor.matmul(ps[:, v * MMW:(v + 1) * MMW], lhsT,
                                         normT[:, lo:lo + MMW],
                                         start=True, stop=True)
                    nc.scalar.copy(sbc[:, w * CHUNK:(w + 1) * CHUNK], ps[:])
                nc.vector.max(cand[:, c * 8:(c + 1) * 8], sbc[:])
                nc.vector.max_index(cpos[:, c * 8:(c + 1) * 8],
                                    cand[:, c * 8:(c + 1) * 8], sbc[:])
                if c == s2at and pending is not None:
                    pend_b = emit_stage2_a(pending)
                    pending = None
                elif c == s2bt and pend_b is not None:
                    emit_stage2_b(pend_b)
                    pend_b = None
            if pending is not None:
                pend_b = emit_stage2_a(pending)
                pending = None
            if pend_b is not None and m == ROWTILES - 1:
                pass
            pending = (m, cand3, cand, cpos)
        pend_b2 = emit_stage2_a(pending)
        if pend_b is not None:
            emit_stage2_b(pend_b, eng=nc.vector)
        emit_stage2_b(pend_b2, eng=nc.vector)

    nc.compile()
    return nc


_NC_CACHE = None
LAST_EXEC_TIME_NS = None


def kernel(embeddings: np.ndarray) -> tuple[np.ndarray, np.ndarray]:
    global _NC_CACHE, LAST_EXEC_TIME_NS
    emb = np.ascontiguousarray(np.asarray(embeddings, dtype=np.float32))
    assert emb.shape == (N, D), emb.shape

    if _NC_CACHE is None:
        _NC_CACHE = _build()
    nc = _NC_CACHE

    in_maps = [{"emb": emb, "slab": emb[i * R:(i + 1) * R].copy()}
               for i in range(NC)]
    trace = os.environ.get("TOPK_TRACE", "0") == "1"
    kwargs = {}
    if trace:
        import tempfile
        kwargs = {"trace": True, "tmpdir": tempfile.mkdtemp(prefix="topk_nt_")}
    res = run_bass_kernel_spmd(nc, in_maps, core_ids=list(range(NC)),
                               **kwargs)
    LAST_EXEC_TIME_NS = res.exec_time_ns

    vals = np.concatenate([res.results[i]["out_vals"][:, :K]
                           for i in range(NC)], 0).astype(np.float32)
    idx = np.concatenate([res.results[i]["out_idx"][:, :K]
                          for i in range(NC)], 0).astype(np.int32)
    flag = np.concatenate([res.results[i]["out_flag"][:, 0]
                           for i in range(NC)], 0)

    frows = np.where(flag > 0)[0]
    if len(frows):
        # exact host recompute for rows whose chunked filter was insufficient
        ssq = np.maximum((emb ** 2).sum(-1, keepdims=True),
                         np.float32(1e-12))
        nrm = (emb / np.sqrt(ssq)).astype(np.float32)
        srows = (nrm[frows] @ nrm.T).astype(np.float32)
        order = np.argsort(-srows, axis=1, kind="stable")[:, :K]
        vals[frows] = np.take_along_axis(srows, order, axis=1)
        idx[frows] = order.astype(np.int32)

    return vals, idx


# revision 32
# speedup vs baseline: 1.0086x; 1.0086x over previous
"""Trainium2 Bass kernel for nn_AdaptiveAdjacency: cosine-similarity top-k.

kernel(embeddings: [16384, 128] f32) -> (values [16384, 20] f32,
                                         indices [16384, 20] int32)

Strategy (8 NeuronCores, SPMD): embeddings are replicated to every core;
core i computes rows [2048*i, 2048*(i+1)) of the similarity matrix against
all 16384 columns in fp32 on the TensorEngine (chunks of 512 columns into
PSUM), and selects per-row top-k on-device:
  - per 512-chunk top-8 values (VectorE MAX8) + in-chunk positions
    (FIND_INDEX8) form 256 exact-f32 candidates per row,
  - a 3-round max8/max_index/match_replace merge picks the top-24 with
    jax.lax.top_k tie semantics (descending value, ascending index),
  - two GPSIMD per-partition local_scatters convert candidate positions
    into value-ranked global indices without any host-side selection,
  - a per-row flag marks rows where one chunk held >= 9 of the row's
    top-20 (candidate filter insufficient); the host recomputes exactly
    those rows (probability ~1e-7 per chunk => ~0 rows expected).
"""

import os
from contextlib import ExitStack

import numpy as np

import concourse.bass as bass
import concourse.mybir as mybir
from concourse import bacc
from concourse.tile import TileContext
from concourse.masks import make_identity
from concourse.bass_utils import run_bass_kernel_spmd

F32 = mybir.dt.float32
U16 = mybir.dt.uint16
I16 = mybir.dt.int16

N = 16384
D = 128
NC = 8
R = N // NC  # rows per core
K = 20
KPAD = 24
CHUNK = 512
MMW = 512
WIN = 1024  # DVE top-8 window (SBUF); >=9-of-top-20 per window falls back
IMM_LO = -3.0


def _build(num_devices=NC, mm_bufs=6, cand_bufs=3, stage2_at=2):
    NCHUNK = N // CHUNK
    NWIN = N // WIN
    WPC = WIN // CHUNK
    NCAND = NWIN * 8
    assert NCAND >= KPAD
    ROWTILES = R // 128
    NTILE = N // 128
    TPG = 8
    DUMP = NCAND + 32

    nc = bacc.Bacc("TRN2", target_bir_lowering=False, debug=False,
                   num_devices=num_devices)
    emb = nc.dram_tensor("emb", [N, D], F32, kind="ExternalInput").ap()
    slab = nc.dram_tensor("slab", [R, D], F32, kind="ExternalInput").ap()
    out_vals = nc.dram_tensor("out_vals", [R, KPAD], F32,
                              kind="ExternalOutput").ap()
    out_idx = nc.dram_tensor("out_idx", [R, KPAD], U16,
                             kind="ExternalOutput").ap()
    out_flag = nc.dram_tensor("out_flag", [R, 1], F32,
                              kind="ExternalOutput").ap()

    with TileContext(nc) as tc, ExitStack() as ctx:
        const_pool = ctx.enter_context(tc.tile_pool(name="const", bufs=1))
        big_pool = ctx.enter_context(tc.tile_pool(name="big", bufs=1))
        norm_pool = ctx.enter_context(tc.tile_pool(name="norm", bufs=10))
        tp_psum = ctx.enter_context(tc.tile_pool(name="tpps", bufs=2,
                                                 space="PSUM"))
        mm_psum = ctx.enter_context(tc.tile_pool(name="mmps", bufs=mm_bufs,
                                                 space="PSUM"))
        cand_pool = ctx.enter_context(tc.tile_pool(name="cand",
                                                   bufs=cand_bufs))
        sbc_pool = ctx.enter_context(tc.tile_pool(name="sbc", bufs=8))
        s2_pool = ctx.enter_context(tc.tile_pool(name="s2", bufs=2))

        ident = const_pool.tile([128, 128], F32)
        make_identity(nc, ident[:])
        offs16 = const_pool.tile([128, NCAND], U16)
        nc.gpsimd.iota(offs16[:], pattern=[[WIN, NWIN], [0, 8]],
                       base=0, channel_multiplier=0)
        dump16 = const_pool.tile([128, NCAND], U16)
        nc.gpsimd.iota(dump16[:], pattern=[[1, NCAND]], base=32,
                       channel_multiplier=0)
        dump_f = const_pool.tile([128, NCAND], F32)
        nc.vector.tensor_copy(dump_f[:], dump16[:])
        iota24 = const_pool.tile([128, KPAD], U16)
        nc.gpsimd.iota(iota24[:], pattern=[[1, KPAD]], base=1,
                       channel_multiplier=0)

        normT = big_pool.tile([128, N], F32)
        rowT = big_pool.tile([128, R], F32)

        def norm_group(src_ap, g, ntiles_left, dstT):
            n = min(TPG, ntiles_left)
            ssq = norm_pool.tile([128, TPG], F32, tag="ssq")
            et4 = norm_pool.tile([128, TPG, D], F32, tag="et4")
            src3 = src_ap.rearrange("(t p) d -> p t d", p=128)
            nc.sync.dma_start(out=et4[:, :n, :],
                              in_=src3[:, g * TPG:g * TPG + n, :])
            ets = []
            for j in range(n):
                et = et4[:, j, :]
                trash = norm_pool.tile([128, D], F32, tag="trash")
                if j % 2 == 0:
                    nc.scalar.activation(trash[:], et,
                                         mybir.ActivationFunctionType.Square,
                                         accum_out=ssq[:, j:j + 1])
                else:
                    nc.vector.tensor_tensor(out=trash[:], in0=et,
                                            in1=et,
                                            op=mybir.AluOpType.mult)
                    nc.vector.reduce_sum(ssq[:, j:j + 1], trash[:],
                                         axis=mybir.AxisListType.X)
                ets.append(et)
            rt = norm_pool.tile([128, TPG], F32, tag="rt")
            nc.scalar.sqrt(rt[:, :n], ssq[:, :n])
            nc.vector.reciprocal(rt[:, :n], rt[:, :n])
            # one Newton step for rsqrt: r' = 0.5*r*(3 - ssq*r^2)
            r2 = norm_pool.tile([128, TPG], F32, tag="r2")
            nc.vector.tensor_tensor(out=r2[:, :n], in0=rt[:, :n],
                                    in1=rt[:, :n], op=mybir.AluOpType.mult)
            nc.vector.tensor_tensor(out=r2[:, :n], in0=ssq[:, :n],
                                    in1=r2[:, :n], op=mybir.AluOpType.mult)
            # out = (b - 3) * r, then * -0.5  =>  0.5*r*(3-b)
            nc.vector.scalar_tensor_tensor(
                out=r2[:, :n], in0=r2[:, :n], scalar=3.0, in1=rt[:, :n],
                op0=mybir.AluOpType.subtract, op1=mybir.AluOpType.mult)
            nc.vector.tensor_scalar_mul(rt[:, :n], r2[:, :n], -0.5)
            for j in range(n):
                i = g * TPG + j
                nt = norm_pool.tile([128, D], F32, tag=f"nt{j}")
                if j % 2 == 0:
                    nc.scalar.mul(nt[:], ets[j], rt[:, j:j + 1])
                else:
                    nc.vector.tensor_scalar_mul(nt[:], ets[j],
                                                rt[:, j:j + 1])
                pt = tp_psum.tile([128, 128], F32)
                nc.tensor.transpose(pt[:], nt[:], ident[:])
                if j % 2 == 0:
                    nc.scalar.copy(dstT[:, i * 128:(i + 1) * 128], pt[:])
                else:
                    nc.vector.tensor_copy(dstT[:, i * 128:(i + 1) * 128],
                                          pt[:])

        for g in range((R // 128 + TPG - 1) // TPG):
            norm_group(slab, g, ROWTILES - g * TPG, rowT)
        for g in range((NTILE + TPG - 1) // TPG):
            norm_group(emb, g, NTILE - g * TPG, normT)

        def emit_stage2_a(st):
            (m, cand3, cand, cpos) = st
            gidx = s2_pool.tile([128, NCAND], U16, tag="gidx")
            nc.vector.tensor_tensor(out=gidx[:], in0=cpos, in1=offs16[:],
                                    op=mybir.AluOpType.add)
            v8 = s2_pool.tile([128, NWIN], F32, tag="v8")
            nc.vector.tensor_copy(v8[:], cand3[:, :, 7])
            wv = s2_pool.tile([128, KPAD], F32, tag="wv")
            wp = s2_pool.tile([128, KPAD], U16, tag="wp")
            for r in range(KPAD // 8):
                sl = slice(r * 8, (r + 1) * 8)
                nc.vector.max(wv[:, sl], cand)
                nc.vector.max_index(wp[:, sl], wv[:, sl], cand)
                if r < KPAD // 8 - 1:
                    nc.vector.match_replace(cand, wv[:, sl], cand, IMM_LO)
            mv8 = s2_pool.tile([128, 1], F32, tag="mv8")
            nc.vector.tensor_reduce(mv8[:], v8[:], axis=mybir.AxisListType.X,
                                    op=mybir.AluOpType.max)
            flag = s2_pool.tile([128, 1], F32, tag="flag")
            nc.vector.tensor_tensor(out=flag[:], in0=mv8[:],
                                    in1=wv[:, K - 1:K],
                                    op=mybir.AluOpType.is_ge)
            vr16 = s2_pool.tile([128, NCAND], U16, tag="vr16")
            nc.gpsimd.local_scatter(vr16[:], iota24[:], wp[:].bitcast(I16),
                                    channels=128, num_elems=NCAND,
                                    num_idxs=KPAD)
            return (m, gidx, wv, flag, vr16)

        def emit_stage2_b(stb, eng=None):
            (m, gidx, wv, flag, vr16) = stb
            eng = eng or nc.gpsimd
            vr_f = s2_pool.tile([128, NCAND], F32, tag="vr_f")
            eng.tensor_copy(vr_f[:], vr16[:])
            marked = s2_pool.tile([128, NCAND], F32, tag="marked")
            eng.tensor_scalar(out=marked[:], in0=vr_f[:], scalar1=0.0,
                                    scalar2=None,
                                    op0=mybir.AluOpType.is_gt)
            t0 = s2_pool.tile([128, NCAND], F32, tag="t0")
            eng.tensor_tensor(out=t0[:], in0=marked[:], in1=dump_f[:],
                                    op=mybir.AluOpType.mult)
            eng.tensor_tensor(out=t0[:], in0=dump_f[:], in1=t0[:],
                                    op=mybir.AluOpType.subtract)
            eng.tensor_tensor(out=vr_f[:], in0=vr_f[:], in1=t0[:],
                                    op=mybir.AluOpType.add)
            idx2 = s2_pool.tile([128, NCAND], I16, tag="idx2")
            eng.tensor_copy(idx2[:], vr_f[:])
            gbr = s2_pool.tile([128, DUMP], U16, tag="gbr")
            nc.gpsimd.local_scatter(gbr[:], gidx[:], idx2[:],
                                    channels=128, num_elems=DUMP,
                                    num_idxs=NCAND)
            rs = slice(m * 128, (m + 1) * 128)
            nc.sync.dma_start(out=out_vals[rs, :], in_=wv[:])
            nc.sync.dma_start(out=out_idx[rs, :], in_=gbr[:, 1:KPAD + 1])
            nc.sync.dma_start(out=out_flag[rs, :], in_=flag[:])

        s2at = min(stage2_at, NWIN - 1)
        s2bt = min(stage2_at + 4, NWIN - 1)
        pending = None
        pend_b = None
        for m in range(ROWTILES):
            lhsT = rowT[:, m * 128:(m + 1) * 128]
            cand3 = cand_pool.tile([128, NWIN, 8], F32, tag="cand")
            cpos3 = cand_pool.tile([128, NWIN, 8], U16, tag="cpos")
            cand = cand3[:].rearrange("p c e -> p (c e)")
            cpos = cpos3[:].rearrange("p c e -> p (c e)")
            for c in range(NWIN):
                sbc = sbc_pool.tile([128, WIN], F32, tag="sbc")
                for w in range(WPC):
                    ps = mm_psum.tile([128, CHUNK], F32)
                    for v in range(CHUNK // MMW):
                        lo = c * WIN + w * CHUNK + v * MMW
                        nc.tensor.matmul(ps[:, v * MMW:(v + 1) * MMW], lhsT,
                                         normT[:, lo:lo + MMW],
                                         start=True, stop=True)
                    nc.scalar.copy(sbc[:, w * CHUNK:(w + 1) * CHUNK], ps[:])
                nc.vector.max(cand[:, c * 8:(c + 1) * 8], sbc[:])
                nc.vector.max_index(cpos[:, c * 8:(c + 1) * 8],
                                    cand[:, c * 8:(c + 1) * 8], sbc[:])
                if c == s2at and pending is not None:
                    pend_b = emit_stage2_a(pending)
                    pending = None
                elif c == s2bt and pend_b is not None:
                    emit_stage2_b(pend_b)
                    pend_b = None
            if pending is not None:
                pend_b = emit_stage2_a(pending)
                pending = None
            if pend_b is not None and m == ROWTILES - 1:
                pass
            pending = (m, cand3, cand, cpos)
        pend_b2 = emit_stage2_a(pending)
        if pend_b is not None:
            emit_stage2_b(pend_b, eng=nc.vector)
        emit_stage2_b(pend_b2, eng=nc.vector)

    nc.compile()
    return nc


_NC_CACHE = None
LAST_EXEC_TIME_NS = None


def kernel(embeddings: np.ndarray) -> tuple[np.ndarray, np.ndarray]:
    global _NC_CACHE, LAST_EXEC_TIME_NS
    emb = np.ascontiguousarray(np.asarray(embeddings, dtype=np.float32))
    assert emb.shape == (N, D), emb.shape

    if _NC_CACHE is None:
        _NC_CACHE = _build()
    nc = _NC_CACHE

    in_maps = [{"emb": emb, "slab": emb[i * R:(i + 1) * R].copy()}
               for i in range(NC)]
    trace = os.environ.get("TOPK_TRACE", "0") == "1"
    kwargs = {}
    if trace:
        import tempfile
        kwargs = {"trace": True, "tmpdir": tempfile.mkdtemp(prefix="topk_nt_")}
    res = run_bass_kernel_spmd(nc, in_maps, core_ids=list(range(NC)),
                               **kwargs)
    LAST_EXEC_TIME_NS = res.exec_time_ns

    vals = np.concatenate([res.results[i]["out_vals"][:, :K]
                           for i in range(NC)], 0).astype(np.float32)
    idx = np.concatenate([res.results[i]["out_idx"][:, :K]
                          for i in range(NC)], 0).astype(np.int32)
    flag = np.concatenate([res.results[i]["out_flag"][:, 0]
                           for i in range(NC)], 0)

    frows = np.where(flag > 0)[0]
    if len(frows):
        # exact host recompute for rows whose chunked filter was insufficient
        ssq = np.maximum((emb ** 2).sum(-1, keepdims=True),
                         np.float32(1e-12))
        nrm = (emb / np.sqrt(ssq)).astype(np.float32)
        srows = (nrm[frows] @ nrm.T).astype(np.float32)
        order = np.argsort(-srows, axis=1, kind="stable")[:, :K]
        vals[frows] = np.take_along_axis(srows, order, axis=1)
        idx[frows] = order.astype(np.int32)

    return vals, idx


# revision 35
# speedup vs baseline: 1.2113x; 1.2009x over previous
"""Trainium2 Bass kernel for nn_AdaptiveAdjacency: cosine-similarity top-k.

kernel(embeddings: [16384, 128] f32) -> (values [16384, 20] f32,
                                         indices [16384, 20] int32)

Strategy (8 NeuronCores, SPMD): embeddings are replicated to every core;
core i computes rows [2048*i, 2048*(i+1)) of the similarity matrix against
all 16384 columns in fp32 on the TensorEngine (chunks of 512 columns into
PSUM), and selects per-row top-k on-device:
  - per 512-chunk top-8 values (VectorE MAX8) + in-chunk positions
    (FIND_INDEX8) form 256 exact-f32 candidates per row,
  - a 3-round max8/max_index/match_replace merge picks the top-24 with
    jax.lax.top_k tie semantics (descending value, ascending index),
  - two GPSIMD per-partition local_scatters convert candidate positions
    into value-ranked global indices without any host-side selection,
  - a per-row flag marks rows where one chunk held >= 9 of the row's
    top-20 (candidate filter insufficient); the host recomputes exactly
    those rows (probability ~1e-7 per chunk => ~0 rows expected).
"""

import os
from contextlib import ExitStack

import numpy as np

import concourse.bass as bass
import concourse.mybir as mybir
from concourse import bacc
from concourse.tile import TileContext
from concourse.masks import make_identity
from concourse.bass_utils import run_bass_kernel_spmd

F32 = mybir.dt.float32
U16 = mybir.dt.uint16
I16 = mybir.dt.int16

N = 16384
D = 128
NC = 8
R = N // NC  # rows per core
K = 20
KPAD = 24
CHUNK = 512
MMW = 512
WIN = 1024  # DVE top-8 window (SBUF); >=9-of-top-20 per window falls back
IMM_LO = -3.0


def _build(num_devices=NC, mm_bufs=6, cand_bufs=3, stage2_at=2):
    NCHUNK = N // CHUNK
    NWIN = N // WIN
    WPC = WIN // CHUNK
    NCAND = NWIN * 8
    assert NCAND >= KPAD
    ROWTILES = R // 128
    NTILE = N // 128
    TPG = 8
    DUMP = NCAND + 32

    nc = bacc.Bacc("TRN2", target_bir_lowering=False, debug=False,
                   num_devices=num_devices)
    emb = nc.dram_tensor("emb", [N, D], F32, kind="ExternalInput").ap()
    slab = nc.dram_tensor("slab", [R, D], F32, kind="ExternalInput").ap()
    out_vals = nc.dram_tensor("out_vals", [R, KPAD], F32,
                              kind="ExternalOutput").ap()
    out_idx = nc.dram_tensor("out_idx", [R, KPAD], U16,
                             kind="ExternalOutput").ap()
    out_flag = nc.dram_tensor("out_flag", [R, 1], F32,
                              kind="ExternalOutput").ap()

    with TileContext(nc) as tc, ExitStack() as ctx:
        const_pool = ctx.enter_context(tc.tile_pool(name="const", bufs=1))
        big_pool = ctx.enter_context(tc.tile_pool(name="big", bufs=1))
        norm_pool = ctx.enter_context(tc.tile_pool(name="norm", bufs=10))
        tp_psum = ctx.enter_context(tc.tile_pool(name="tpps", bufs=2,
                                                 space="PSUM"))
        mm_psum = ctx.enter_context(tc.tile_pool(name="mmps", bufs=mm_bufs,
                                                 space="PSUM"))
        cand_pool = ctx.enter_context(tc.tile_pool(name="cand",
                                                   bufs=cand_bufs))
        sbc_pool = ctx.enter_context(tc.tile_pool(name="sbc", bufs=8))
        s2_pool = ctx.enter_context(tc.tile_pool(name="s2", bufs=2))

        ident = const_pool.tile([128, 128], F32)
        make_identity(nc, ident[:])
        offs16 = const_pool.tile([128, NCAND], U16)
        nc.gpsimd.iota(offs16[:], pattern=[[WIN, NWIN], [0, 8]],
                       base=0, channel_multiplier=0)
        dump16 = const_pool.tile([128, NCAND], U16)
        nc.gpsimd.iota(dump16[:], pattern=[[1, NCAND]], base=32,
                       channel_multiplier=0)
        dump_f = const_pool.tile([128, NCAND], F32)
        nc.vector.tensor_copy(dump_f[:], dump16[:])
        iota24 = const_pool.tile([128, KPAD], U16)
        nc.gpsimd.iota(iota24[:], pattern=[[1, KPAD]], base=1,
                       channel_multiplier=0)

        normT = big_pool.tile([128, N], F32)
        rowT = big_pool.tile([128, R], F32)

        def norm_group(src_ap, g, ntiles_left, dstT):
            n = min(TPG, ntiles_left)
            ssq = norm_pool.tile([128, TPG], F32, tag="ssq")
            et4 = norm_pool.tile([128, TPG, D], F32, tag="et4")
            src3 = src_ap.rearrange("(t p) d -> p t d", p=128)
            nc.sync.dma_start(out=et4[:, :n, :],
                              in_=src3[:, g * TPG:g * TPG + n, :])
            ets = []
            for j in range(n):
                et = et4[:, j, :]
                trash = norm_pool.tile([128, D], F32, tag="trash")
                if j % 2 == 0:
                    nc.scalar.activation(trash[:], et,
                                         mybir.ActivationFunctionType.Square,
                                         accum_out=ssq[:, j:j + 1])
                else:
                    nc.vector.tensor_tensor(out=trash[:], in0=et,
                                            in1=et,
                                            op=mybir.AluOpType.mult)
                    nc.vector.reduce_sum(ssq[:, j:j + 1], trash[:],
                                         axis=mybir.AxisListType.X)
                ets.append(et)
            rt = norm_pool.tile([128, TPG], F32, tag="rt")
            nc.scalar.sqrt(rt[:, :n], ssq[:, :n])
            nc.vector.reciprocal(rt[:, :n], rt[:, :n])
            # one Newton step for rsqrt: r' = 0.5*r*(3 - ssq*r^2)
            r2 = norm_pool.tile([128, TPG], F32, tag="r2")
            nc.vector.tensor_tensor(out=r2[:, :n], in0=rt[:, :n],
                                    in1=rt[:, :n], op=mybir.AluOpType.mult)
            nc.vector.tensor_tensor(out=r2[:, :n], in0=ssq[:, :n],
                                    in1=r2[:, :n], op=mybir.AluOpType.mult)
            # out = (b - 3) * r, then * -0.5  =>  0.5*r*(3-b)
            nc.vector.scalar_tensor_tensor(
                out=r2[:, :n], in0=r2[:, :n], scalar=3.0, in1=rt[:, :n],
                op0=mybir.AluOpType.subtract, op1=mybir.AluOpType.mult)
            nc.vector.tensor_scalar_mul(rt[:, :n], r2[:, :n], -0.5)
            for j in range(n):
                i = g * TPG + j
                nt = norm_pool.tile([128, D], F32, tag=f"nt{j}")
                if j % 2 == 0:
                    nc.scalar.mul(nt[:], ets[j], rt[:, j:j + 1])
                else:
                    nc.vector.tensor_scalar_mul(nt[:], ets[j],
                                                rt[:, j:j + 1])
                pt = tp_psum.tile([128, 128], F32)
                nc.tensor.transpose(pt[:], nt[:], ident[:])
                if j % 2 == 0:
                    nc.scalar.copy(dstT[:, i * 128:(i + 1) * 128], pt[:])
                else:
                    nc.vector.tensor_copy(dstT[:, i * 128:(i + 1) * 128],
                                          pt[:])

        for g in range((R // 128 + TPG - 1) // TPG):
            norm_group(slab, g, ROWTILES - g * TPG, rowT)
        for g in range((NTILE + TPG - 1) // TPG):
            norm_group(emb, g, NTILE - g * TPG, normT)

        def emit_stage2_a(st):
            (m, cand3, cand, cpos) = st
            gidx = s2_pool.tile([128, NCAND], U16, tag="gidx")
            nc.vector.tensor_tensor(out=gidx[:], in0=cpos, in1=offs16[:],
                                    op=mybir.AluOpType.add)
            v8 = s2_pool.tile([128, NWIN], F32, tag="v8")
            nc.vector.tensor_copy(v8[:], cand3[:, :, 7])
            wv = s2_pool.tile([128, KPAD], F32, tag="wv")
            wp = s2_pool.tile([128, KPAD], U16, tag="wp")
            for r in range(KPAD // 8):
                sl = slice(r * 8, (r + 1) * 8)
                nc.vector.max(wv[:, sl], cand)
                nc.vector.max_index(wp[:, sl], wv[:, sl], cand)
                if r < KPAD // 8 - 1:
                    nc.vector.match_replace(cand, wv[:, sl], cand, IMM_LO)
            mv8 = s2_pool.tile([128, 1], F32, tag="mv8")
            nc.vector.tensor_reduce(mv8[:], v8[:], axis=mybir.AxisListType.X,
                                    op=mybir.AluOpType.max)
            flag = s2_pool.tile([128, 1], F32, tag="flag")
            nc.vector.tensor_tensor(out=flag[:], in0=mv8[:],
                                    in1=wv[:, K - 1:K],
                                    op=mybir.AluOpType.is_ge)
            vr16 = s2_pool.tile([128, NCAND], U16, tag="vr16")
            nc.gpsimd.local_scatter(vr16[:], iota24[:], wp[:].bitcast(I16),
                                    channels=128, num_elems=NCAND,
                                    num_idxs=KPAD)
            return (m, gidx, wv, flag, vr16)

        def emit_stage2_b(stb, eng=None):
            (m, gidx, wv, flag, vr16) = stb
            eng = eng or nc.gpsimd
            vr_f = s2_pool.tile([128, NCAND], F32, tag="vr_f")
            eng.tensor_copy(vr_f[:], vr16[:])
            marked = s2_pool.tile([128, NCAND], F32, tag="marked")
            eng.tensor_scalar(out=marked[:], in0=vr_f[:], scalar1=0.0,
                                    scalar2=None,
                                    op0=mybir.AluOpType.is_gt)
            t0 = s2_pool.tile([128, NCAND], F32, tag="t0")
            eng.tensor_tensor(out=t0[:], in0=marked[:], in1=dump_f[:],
                                    op=mybir.AluOpType.mult)
            eng.tensor_tensor(out=t0[:], in0=dump_f[:], in1=t0[:],
                                    op=mybir.AluOpType.subtract)
            eng.tensor_tensor(out=vr_f[:], in0=vr_f[:], in1=t0[:],
                                    op=mybir.AluOpType.add)
            idx2 = s2_pool.tile([128, NCAND], I16, tag="idx2")
            eng.tensor_copy(idx2[:], vr_f[:])
            gbr = s2_pool.tile([128, DUMP], U16, tag="gbr")
            nc.gpsimd.local_scatter(gbr[:], gidx[:], idx2[:],
                                    channels=128, num_elems=DUMP,
                                    num_idxs=NCAND)
            rs = slice(m * 128, (m + 1) * 128)
            nc.sync.dma_start(out=out_vals[rs, :], in_=wv[:])
            nc.sync.dma_start(out=out_idx[rs, :], in_=gbr[:, 1:KPAD + 1])
            nc.sync.dma_start(out=out_flag[rs, :], in_=flag[:])

        s2at = min(stage2_at, NWIN - 1)
        s2bt = min(stage2_at + 4, NWIN - 1)
        pending = None
        pend_b = None
        for m in range(ROWTILES):
            lhsT = rowT[:, m * 128:(m + 1) * 128]
            cand3 = cand_pool.tile([128, NWIN, 8], F32, tag="cand")
            cpos3 = cand_pool.tile([128, NWIN, 8], U16, tag="cpos")
            cand = cand3[:].rearrange("p c e -> p (c e)")
            cpos = cpos3[:].rearrange("p c e -> p (c e)")
            for c in range(NWIN):
                sbc = sbc_pool.tile([128, WIN], F32, tag="sbc")
                for w in range(WPC):
                    ps = mm_psum.tile([128, CHUNK], F32)
                    for v in range(CHUNK // MMW):
                        lo = c * WIN + w * CHUNK + v * MMW
                        nc.tensor.matmul(ps[:, v * MMW:(v + 1) * MMW], lhsT,
                                         normT[:, lo:lo + MMW],
                                         start=True, stop=True)
                    nc.scalar.copy(sbc[:, w * CHUNK:(w + 1) * CHUNK], ps[:])
                nc.vector.max(cand[:, c * 8:(c + 1) * 8], sbc[:])
                nc.vector.max_index(cpos[:, c * 8:(c + 1) * 8],
                                    cand[:, c * 8:(c + 1) * 8], sbc[:])
                if c == s2at and pending is not None:
                    pend_b = emit_stage2_a(pending)
                    pending = None
                elif c == s2bt and pend_b is not None:
                    emit_stage2_b(pend_b)
                    pend_b = None
            if pending is not None:
                pend_b = emit_stage2_a(pending)
                pending = None
            if pend_b is not None and m == ROWTILES - 1:
                pass
            pending = (m, cand3, cand, cpos)
        pend_b2 = emit_stage2_a(pending)
        if pend_b is not None:
            emit_stage2_b(pend_b, eng=nc.vector)
        emit_stage2_b(pend_b2, eng=nc.vector)

    nc.compile()
    return nc


_NC_CACHE = None
LAST_EXEC_TIME_NS = None


def kernel(embeddings: np.ndarray) -> tuple[np.ndarray, np.ndarray]:
    global _NC_CACHE, LAST_EXEC_TIME_NS
    emb = np.ascontiguousarray(np.asarray(embeddings, dtype=np.float32))
    assert emb.shape == (N, D), emb.shape

    if _NC_CACHE is None:
        _NC_CACHE = _build()
    nc = _NC_CACHE

    in_maps = [{"emb": emb, "slab": emb[i * R:(i + 1) * R].copy()}
               for i in range(NC)]
    trace = os.environ.get("TOPK_TRACE", "0") == "1"
    kwargs = {}
    if trace:
        import tempfile
        kwargs = {"trace": True, "tmpdir": tempfile.mkdtemp(prefix="topk_nt_")}
    res = run_bass_kernel_spmd(nc, in_maps, core_ids=list(range(NC)),
                               **kwargs)
    LAST_EXEC_TIME_NS = res.exec_time_ns

    vals = np.concatenate([res.results[i]["out_vals"][:, :K]
                           for i in range(NC)], 0).astype(np.float32)
    idx = np.concatenate([res.results[i]["out_idx"][:, :K]
                          for i in range(NC)], 0).astype(np.int32)
    flag = np.concatenate([res.results[i]["out_flag"][:, 0]
                           for i in range(NC)], 0)

    frows = np.where(flag > 0)[0]
    if len(frows):
        # exact host recompute for rows whose chunked filter was insufficient
        ssq = np.maximum((emb ** 2).sum(-1, keepdims=True),
                         np.float32(1e-12))
        nrm = (emb / np.sqrt(ssq)).astype(np.float32)
        srows = (nrm[frows] @ nrm.T).astype(np.float32)
        order = np.argsort(-srows, axis=1, kind="stable")[:, :K]
        vals[frows] = np.take_along_axis(srows, order, axis=1)
        idx[frows] = order.astype(np.int32)

    return vals, idx
